# revision 39
# baseline (speedup 1.0000x reference)
"""Trainium2 Bass kernel for DMRNet-style GNN message passing (8 NeuronCores).

Sharding: row-parallel over query visits i. Core c owns the 32 visits
I_c = [16c, 16c+16) u [128+16c, 144+16c): every core gets exactly 16
one-j-tile rows (i<128) and 16 two-j-tile rows (i>=128), so the tril pairwise
work is balanced and the SPMD program is identical on all cores.

Per core: gather embeddings + build multihots for OWN visits only, AllGather a
packed [32, 559] payload (patient_rep | drug_mh | combo_mh), rebuild the
visit-shifted j-ordered tensors, then run the tril-masked pairwise MLP for its
own rows. No cross-device reduction needed.

Perf notes:
- ~1.9us fixed cost per dma_start => all small weights ship pre-packed in one
  [128, W] input, indices in one int32 pack, outputs buffered in SBUF and
  written with one DMA per tensor. HW indirect DMA takes ONE index per
  partition, so token embeddings use 24 per-tile gathers; multihots are built
  with iota-compares on DVE instead of one-hot gathers.
- emission order = schedule priority: payload-critical work (token means,
  multihots) is emitted first so the AllGather fires early; the token tanh-MLP
  path is emitted late so it fills the collective + pair-loop idle gaps.
- PSUM accumulation groups never share a bank (start=True zeroes the whole
  2KB bank region).
"""

import os

import numpy as np

# A process that exits after running collectives can leave the NeuronCores in
# an unrecoverable state for the next process; request a core reset at NRT
# init (no-op on a healthy device).
os.environ.setdefault("NEURON_RT_RESET_CORES", "1")

# ---------------- problem constants (hardcoded) ----------------
V, L, M, K = 256, 48, 24, 10
DV, PV, CV = 2000, 1500, 300
E, MD, MEDV, C = 64, 128, 131, 300
NCORES = 8
NI = 32          # own i rows per core
NB = 16          # rows per mirrored half-block
PAY = MD + MEDV + C   # 128 + 131 + 300 = 559
MP = MEDV + 1         # 132

_CACHE = {}

_WL = {}
_BL = {}


_CL = {}


def _clayout():
    if _CL:
        return _CL
    o = 0

    def add(name, rows, cols):
        nonlocal o
        _CL[name] = (o, rows, cols)
        o += cols

    add("seg48", 128, 12 * NI)
    add("masksT", 128, NI)
    add("iota131", NI, MEDV)
    add("iota300", NI, C)
    add("ids_dr", NI, M)
    add("ids_co", NI, K)
    _CL["__total__"] = (o, 0, 0)
    return _CL


def _wlayout():
    if _WL:
        return _WL
    o = 0

    def add(name, rows, cols):
        nonlocal o
        _WL[name] = (o, rows, cols)
        o += cols

    add("dl2_w1", 64, 128); add("dl2_w2", 128, 64)
    add("pl2_w1", 64, 128); add("pl2_w2", 128, 64)
    add("dmb", 64, MEDV); add("pmb", 64, MEDV); add("med_block", 128, MEDV)
    add("lch_w1a", 128, 128); add("lch_w1b", 128, 128); add("lch_w2", 128, 128)
    add("cm_w1", 128, 128); add("cm_w2", 128, CV - 1)
    add("pmc_w1t", 128, 256); add("pmc_w1b", 128, 256)
    add("pmc_w2a", 128, MEDV); add("pmc_w2b", 128, MEDV)
    add("clc_w1_0", 128, MEDV); add("clc_w1_1", 3, MEDV)
    add("clc_w1_2", 128, MEDV); add("clc_w1_3", 128, MEDV); add("clc_w1_4", 44, MEDV)
    add("clc_w2_0", 128, MEDV); add("clc_w2_1", 3, MEDV)
    add("pmc_b1", 128, 2)
    _WL["__total__"] = (o, 0, 0)
    return _WL


def _blayout():
    if _BL:
        return _BL
    o = 0

    def add(name, cols):
        nonlocal o
        _BL[name] = (o, cols)
        o += cols

    add("dl2_b1", 128); add("dl2_b2", 64)
    add("pl2_b1", 128); add("pl2_b2", 64)
    add("clc_b1", MEDV); add("clc_b2", MEDV)
    add("lch_b1", 128); add("lch_b2", 128)
    add("pmc_b2", MEDV)
    add("cm_b1", 128); add("cm_b2", CV - 1)
    _BL["__total__"] = (o, 0)
    return _BL


def _core_rows(c):
    # 16 one-j-tile rows (i<128) + 16 two-j-tile rows (i>=128) per core:
    # perfectly balanced under tile-quantized pairwise cost, and the AllGather
    # output stays ascending so the shifted j-order loads are clean APs.
    return list(range(NB * c, NB * c + NB)) + list(range(128 + NB * c, 128 + NB * c + NB))


def _seg_cols(n_rows, per, scale):
    """[128, ntiles*NI]: token g=128t+p contributes `scale` to visit g//per."""
    import math
    n_tiles = math.ceil(n_rows / 128)
    seg = np.zeros((128, n_tiles * NI), np.float32)
    for g in range(n_rows):
        seg[g % 128, (g // 128) * NI + g // per] = scale
    return seg


def _build_nc():
    import concourse.bass as bass
    import concourse.bacc as bacc
    import concourse.mybir as mybir
    import concourse.tile as tile
    from concourse.masks import make_identity

    F32 = mybir.dt.float32
    BF16 = mybir.dt.bfloat16
    I32 = mybir.dt.int32
    TANH = mybir.ActivationFunctionType.Tanh
    SIG = mybir.ActivationFunctionType.Sigmoid
    WL = _wlayout()
    BLL = _blayout()
    CL = _clayout()

    nc = bacc.Bacc("TRN2", target_bir_lowering=False, debug=False, num_devices=NCORES)

    wcrit = nc.dram_tensor("wcrit", [128, CL["__total__"][0]], F32, kind="ExternalInput")
    wpack = nc.dram_tensor("wpack", [128, WL["__total__"][0]], F32, kind="ExternalInput")
    bpack = nc.dram_tensor("bpack", [1, BLL["__total__"][0]], F32, kind="ExternalInput")
    ixpack = nc.dram_tensor("ixpack", [128, 24], I32, kind="ExternalInput")
    # embcat rows: 0..1999 = diag_emb, 2000..3499 = proc_emb
    embcat = nc.dram_tensor("embcat", [DV + PV, E], F32, kind="ExternalInput")

    o_prob = nc.dram_tensor("o_prob", [NI, MP], F32, kind="ExternalOutput")
    o_pair = nc.dram_tensor("o_pair", [MP, NI, MP], F32, kind="ExternalOutput")
    o_combo = nc.dram_tensor("o_combo", [NI, CV], F32, kind="ExternalOutput")
    DBG = {}
    if _CACHE.get("debug"):
        for nm, shp in (("PRown", [NI, MD]), ("pr_own", [NI, MD]), ("Gpack", [128, 2 * MEDV]),
                        ("HIS2", [NI, 2 * MEDV]), ("hm_d", [NI, MD]), ("mh_dr", [NI, MEDV]),
                        ("mh_co", [NI, C]), ("sv", [NI, MEDV]), ("BT0", [128, V]),
                        ("ATb0", [128, NI]), ("hmT", [MD, V])):
            DBG[nm] = nc.dram_tensor("dbg_" + nm, shp, F32, kind="ExternalOutput")

    payload = nc.dram_tensor("payload", [NI, PAY], BF16)
    agbuf = nc.dram_tensor("agbuf", [V, PAY], BF16, addr_space="Shared")

    dma = nc.sync.dma_start

    with tile.TileContext(nc) as tc:
        with (
            tc.tile_pool(name="const", bufs=1) as cp,
            tc.tile_pool(name="work", bufs=4) as wp,
            tc.tile_pool(name="ps4a", bufs=4, space="PSUM") as pa,    # tag: acc (+his)
            tc.tile_pool(name="ps4b", bufs=4, space="PSUM") as pp,    # tag: big
        ):
            ident = cp.tile([128, 128], F32)
            make_identity(nc, ident[:])
            ident_bf = cp.tile([128, 128], BF16)
            nc.vector.tensor_copy(out=ident_bf[:], in_=ident[:])
            ones_r = cp.tile([1, 128], F32)
            nc.vector.memset(ones_r[:], 1.0)
            ones_c = cp.tile([128, 1], F32)
            nc.vector.memset(ones_c[:], 1.0)

            ixt = cp.tile([128, 24], I32)
            dma(out=ixt[:], in_=ixpack[:])
            wc = cp.tile([128, CL["__total__"][0]], F32)
            dma(out=wc[:], in_=wcrit[:])
            wt = cp.tile([128, WL["__total__"][0]], F32)
            nc.scalar.dma_start(out=wt[:], in_=wpack[:])
            bt_ = cp.tile([1, BLL["__total__"][0]], F32)
            nc.scalar.dma_start(out=bt_[:], in_=bpack[:])

            def W(nm):
                if nm in CL:
                    o, r, c = CL[nm]
                    return wc[0:r, o:o + c]
                o, r, c = WL[nm]
                return wt[0:r, o:o + c]

            def B(nm):
                o, c = BLL[nm]
                return bt_[0:1, o:o + c]

            def tr(dst_sb, src_sb):
                p, f = src_sb.shape
                idt = ident if src_sb.dtype == F32 else ident_bf
                ps = pp.tile([128, 128], src_sb.dtype, tag="big")
                nc.tensor.transpose(out=ps[:f, :p], in_=src_sb, identity=idt[:p, :p])
                nc.vector.tensor_copy(out=dst_sb, in_=ps[:f, :p])

            # ---------- phase 1a: gathers + patient_rep means (payload-critical) ----------
            # HW indirect DMA supports ONE index per partition, so the token
            # embeddings come in as 24 per-tile gathers (proc ids host-offset
            # by DV into embcat).
            segs = W("seg48")
            g_all = cp.tile([128, 24 * E], F32)
            for t in range(24):
                nc.gpsimd.indirect_dma_start(
                    out=g_all[:, t * E:(t + 1) * E], out_offset=None, in_=embcat[:],
                    in_offset=bass.IndirectOffsetOnAxis(ap=ixt[:, t:t + 1], axis=0))
            g_alls = {"d": g_all[:, 0:12 * E], "p": g_all[:, 12 * E:24 * E]}
            PRown = cp.tile([NI, MD], F32)
            for ki, pre in enumerate(("d", "p")):
                ga = g_alls[pre]
                accm = pa.tile([NI, E], F32, tag="acc")
                for t in range(12):
                    nc.tensor.matmul(out=accm[:], lhsT=segs[:, t * NI:(t + 1) * NI],
                                     rhs=ga[:, t * E:(t + 1) * E],
                                     start=(t == 0), stop=(t == 11),
                                     skip_group_check=True)
                nc.vector.tensor_copy(out=PRown[:, ki * E:(ki + 1) * E], in_=accm[:])

            # ---------- phase 1b: own multihots via iota-compare on DVE ----------
            mh_own = {}
            for pre, idsnm, iotanm, nm_, ncls in (
                ("dr", "ids_dr", "iota131", M, MEDV),
                ("co", "ids_co", "iota300", K, C),
            ):
                mh = cp.tile([NI, ncls], F32, tag=f"mh_own_{pre}")
                ids = W(idsnm)
                iota = W(iotanm)
                eqs = []
                for m in range(nm_):
                    if m == 0:
                        nc.vector.tensor_scalar(out=mh[:], in0=iota, scalar1=ids[:, 0:1],
                                                scalar2=None, op0=mybir.AluOpType.is_equal)
                    else:
                        eq = wp.tile([NI, ncls], F32, tag=f"mh_eq_{pre}")
                        nc.vector.tensor_scalar(out=eq[:], in0=iota, scalar1=ids[:, m:m + 1],
                                                scalar2=None, op0=mybir.AluOpType.is_equal)
                        nc.vector.tensor_add(out=mh[:], in0=mh[:], in1=eq[:])
                mh_own[pre] = mh

            # ---------- phase 2: payload + AllGather ----------
            pay_sb = cp.tile([NI, PAY], BF16)
            nc.vector.tensor_copy(out=pay_sb[:, 0:MD], in_=PRown[:])
            nc.vector.tensor_copy(out=pay_sb[:, MD:MD + MEDV], in_=mh_own["dr"][:])
            nc.vector.tensor_copy(out=pay_sb[:, MD + MEDV:PAY], in_=mh_own["co"][:])
            dma(out=payload[:], in_=pay_sb[:])
            nc.gpsimd.collective_compute(
                "AllGather", mybir.AluOpType.bypass,
                replica_groups=[list(range(NCORES))],
                ins=[payload[:]], outs=[agbuf[:]])

            # ---------- phase 3: pr path + ATb (independent of collective) ----------
            PRt = cp.tile([MD, NI], F32)
            tr(PRt[:], PRown[:])
            psA = pp.tile([NI, MD], F32, tag="big")
            nc.tensor.matmul(out=psA[:], lhsT=PRt[:], rhs=W("lch_w1a"), start=True, stop=False)
            nc.tensor.matmul(out=psA[:], lhsT=PRt[:], rhs=W("lch_w1b"), start=False, stop=False)
            nc.tensor.matmul(out=psA[:], lhsT=ones_r[:, 0:NI], rhs=B("lch_b1"), start=False, stop=True)
            l1 = wp.tile([NI, MD], F32, tag="l1")
            nc.scalar.activation(out=l1[:], in_=psA[:], func=TANH)
            l1T = wp.tile([MD, NI], F32, tag="l1T")
            tr(l1T[:], l1[:])
            psB = pp.tile([NI, MD], F32, tag="big")
            nc.tensor.matmul(out=psB[:], lhsT=l1T[:], rhs=W("lch_w2"), start=True, stop=False)
            nc.tensor.matmul(out=psB[:], lhsT=ones_r[:, 0:NI], rhs=B("lch_b2"), start=False, stop=True)
            pr_own = cp.tile([NI, MD], F32)
            nc.scalar.activation(out=pr_own[:], in_=psB[:], func=TANH)
            prT = cp.tile([MD, NI], F32)
            tr(prT[:], pr_own[:])

            psAA = pp.tile([NI, 2 * MD], F32, tag="big")
            nc.tensor.matmul(out=psAA[:], lhsT=prT[:], rhs=W("pmc_w1t"), start=True, stop=True)
            A_own = wp.tile([NI, 2 * MD], F32, tag="A_own")
            nc.vector.tensor_copy(out=A_own[:], in_=psAA[:])
            ATb = []
            for kt in range(2):
                at = cp.tile([128, NI], F32, tag=f"atb{kt}")
                ps = pp.tile([128, 128], F32, tag="big")
                nc.tensor.transpose(out=ps[:, 0:NI], in_=A_own[:, 128 * kt:128 * (kt + 1)],
                                    identity=ident[0:NI, 0:NI])
                nc.vector.tensor_scalar_add(out=at[:], in0=ps[:, 0:NI],
                                            scalar1=W("pmc_b1")[:, kt:kt + 1])
                ATb.append(at)

            # ---------- phase 3b: token tanh-MLP path (overlaps the collective) ----------
            hm_sb = {}
            for pre, w1nm, b1nm in (("d", "dl2_w1", "dl2_b1"), ("p", "pl2_w1", "pl2_b1")):
                acch = pa.tile([NI, 2 * E], F32, tag="acc")
                g_all = g_alls[pre]
                for t in range(12):
                    gT = wp.tile([E, 128], F32, tag="tok_gT")
                    tr(gT[:], g_all[:, t * E:(t + 1) * E])
                    h_ps = pp.tile([128, 2 * E], F32, tag="big")
                    nc.tensor.matmul(out=h_ps[:], lhsT=gT[:], rhs=W(w1nm), start=True, stop=False)
                    nc.tensor.matmul(out=h_ps[:], lhsT=ones_r[:], rhs=B(b1nm), start=False, stop=True)
                    htok = wp.tile([128, 2 * E], F32, tag="tok_ht")
                    nc.scalar.activation(out=htok[:], in_=h_ps[:], func=TANH)
                    nc.tensor.matmul(out=acch[:], lhsT=segs[:, t * NI:(t + 1) * NI],
                                     rhs=htok[:], start=(t == 0), stop=(t == 11),
                                     skip_group_check=True)
                h = cp.tile([NI, 2 * E], F32, tag=f"hm_sb_{pre}")
                nc.vector.tensor_copy(out=h[:], in_=acch[:])
                hm_sb[pre] = h

            ps_sq = pa.tile([NI, MEDV], F32, tag="acc")
            nc.tensor.matmul(out=ps_sq[:], lhsT=prT[:], rhs=W("med_block"), start=True, stop=True)
            sq_sb = cp.tile([NI, MEDV], F32)
            nc.vector.tensor_copy(out=sq_sb[:], in_=ps_sq[:])

            ps_tv = pa.tile([NI, MEDV], F32, tag="acc")
            for k, (pre, w2nm, b2nm, mmnm) in enumerate((
                ("d", "dl2_w2", "dl2_b2", "dmb"), ("p", "pl2_w2", "pl2_b2", "pmb"))):
                hmt = wp.tile([2 * E, NI], F32, tag="hmT_tv")
                tr(hmt[:], hm_sb[pre][:])
                ps = pp.tile([NI, E], F32, tag="big")
                nc.tensor.matmul(out=ps[:], lhsT=hmt[:], rhs=W(w2nm), start=True, stop=False)
                nc.tensor.matmul(out=ps[:], lhsT=ones_r[:, 0:NI], rhs=B(b2nm), start=False, stop=True)
                dh = wp.tile([NI, E], F32, tag="dh")
                nc.vector.tensor_copy(out=dh[:], in_=ps[:])
                dhT = wp.tile([E, NI], F32, tag="dhT")
                tr(dhT[:], dh[:])
                nc.tensor.matmul(out=ps_tv[:], lhsT=dhT[:], rhs=W(mmnm),
                                 start=(k == 0), stop=(k == 1), skip_group_check=True)
            tv_sb = cp.tile([NI, MEDV], F32)
            nc.vector.tensor_copy(out=tv_sb[:], in_=ps_tv[:])
            svp = cp.tile([NI, MEDV], F32)
            nc.vector.tensor_add(out=svp[:], in0=sq_sb[:], in1=tv_sb[:])

            # ---------- phase 3c: combo_prob (independent of collective) ----------
            psc = pp.tile([NI, MD], F32, tag="big")
            nc.tensor.matmul(out=psc[:], lhsT=PRt[:], rhs=W("cm_w1"), start=True, stop=False)
            nc.tensor.matmul(out=psc[:], lhsT=ones_r[:, 0:NI], rhs=B("cm_b1"), start=False, stop=True)
            c1 = wp.tile([NI, MD], F32, tag="c1")
            nc.scalar.activation(out=c1[:], in_=psc[:], func=TANH)
            c1T = wp.tile([MD, NI], F32, tag="c1T")
            tr(c1T[:], c1[:])
            psc2 = pp.tile([NI, CV - 1], F32, tag="big")
            nc.tensor.matmul(out=psc2[:], lhsT=c1T[:], rhs=W("cm_w2"), start=True, stop=False)
            nc.tensor.matmul(out=psc2[:], lhsT=ones_r[:, 0:NI], rhs=B("cm_b2"), start=False, stop=True)
            cpd = cp.tile([NI, CV], F32, tag="cpd")
            nc.vector.memset(cpd[:], 0.0)
            nc.scalar.activation(out=cpd[:, 1:CV], in_=psc2[:], func=SIG)
            dma(out=o_combo[:], in_=cpd[:])

            # ---------- phase 4: j-ordered shifted payload, BT, clc/G ----------
            agv = agbuf[:].rearrange("(c k) f -> c k f", c=NCORES)
            PAYJ = []
            for jt in range(2):
                pj = cp.tile([128, PAY], BF16, tag=f"payj{jt}")
                if jt == 0:
                    nc.vector.memset(pj[0:1, :], 0.0)
                    dma(out=pj[1:113, :], in_=agv[0:7, 0:NB, :])
                    dma(out=pj[113:128, :], in_=agbuf[7 * NI:7 * NI + 15, :])
                else:
                    # visit 127 = (c=7 block0, k=15) -> row 0
                    dma(out=pj[0:1, :], in_=agbuf[7 * NI + NB - 1:7 * NI + NB, :])
                    # visits 128..239 = block1 of c=0..6 -> rows 1..112
                    dma(out=pj[1:113, :], in_=agv[0:7, NB:NI, :])
                    # visits 240..254 = (c=7 block1, k=0..14) -> rows 113..127
                    dma(out=pj[113:128, :], in_=agbuf[7 * NI + NB:7 * NI + NB + 15, :])
                PAYJ.append(pj)
            hmT = cp.tile([MD, V], F32)
            DS = [PAYJ[jt][:, MD:MD + MEDV] for jt in range(2)]
            CS = [PAYJ[jt][:, MD + MEDV:PAY] for jt in range(2)]
            for jt in range(2):
                tr(hmT[:, 128 * jt:128 * (jt + 1)], PAYJ[jt][:, 0:MD])

            BTt = []
            for hc in range(2):
                ps = pp.tile([128, V], F32, tag="big")
                nc.tensor.matmul(out=ps[:], lhsT=W("pmc_w1b")[:, 128 * hc:128 * (hc + 1)],
                                 rhs=hmT[:], start=True, stop=True)
                bt = cp.tile([128, V], F32, tag=f"bt{hc}")
                nc.vector.tensor_copy(out=bt[:], in_=ps[:])
                BTt.append(bt)

            Gpack = cp.tile([128, 2 * MEDV], F32)
            for jt in range(2):
                dsT0 = wp.tile([128, 128], F32, tag="dsT0")
                tr(dsT0[:], DS[jt][:, 0:128])
                dsT1 = wp.tile([3, 128], F32, tag="dsT1")
                tr(dsT1[:], DS[jt][:, 128:MEDV])
                csT = []
                for kc, wdt in ((0, 128), (1, 128), (2, 44)):
                    t_ = wp.tile([wdt, 128], F32, tag=f"csT{kc}")
                    tr(t_[:], CS[jt][:, 128 * kc:128 * kc + wdt])
                    csT.append(t_)
                ps1_ = pp.tile([128, MEDV], F32, tag="big")
                lhs_list = [dsT0, dsT1] + csT
                for kc in range(5):
                    nc.tensor.matmul(out=ps1_[:], lhsT=lhs_list[kc][:],
                                     rhs=W(f"clc_w1_{kc}"),
                                     start=(kc == 0), stop=False)
                nc.tensor.matmul(out=ps1_[:], lhsT=ones_r[:], rhs=B("clc_b1"), start=False, stop=True)
                h1 = wp.tile([128, MEDV], F32, tag="clc_h1")
                nc.scalar.activation(out=h1[:], in_=ps1_[:], func=TANH)
                h1T0 = wp.tile([128, 128], F32, tag="h1T0")
                tr(h1T0[:], h1[:, 0:128])
                h1T1 = wp.tile([3, 128], F32, tag="h1T1")
                tr(h1T1[:], h1[:, 128:MEDV])
                ps2_ = pp.tile([128, MEDV], F32, tag="big")
                nc.tensor.matmul(out=ps2_[:], lhsT=h1T0[:], rhs=W("clc_w2_0"), start=True, stop=False)
                nc.tensor.matmul(out=ps2_[:], lhsT=h1T1[:], rhs=W("clc_w2_1"), start=False, stop=False)
                nc.tensor.matmul(out=ps2_[:], lhsT=ones_r[:], rhs=B("clc_b2"), start=False, stop=True)
                av = wp.tile([128, MEDV], F32, tag="clc_av")
                nc.scalar.activation(out=av[:], in_=ps2_[:], func=TANH)
                nc.vector.tensor_add(out=Gpack[:, MEDV * jt:MEDV * (jt + 1)],
                                     in0=av[:], in1=DS[jt])

            # ---------- phase 5: pair loop ----------
            w2bf = cp.tile([128, 2 * MEDV], BF16)
            nc.vector.tensor_copy(out=w2bf[:, 0:MEDV], in_=W("pmc_w2a"))
            nc.vector.tensor_copy(out=w2bf[:, MEDV:2 * MEDV], in_=W("pmc_w2b"))
            b2b2 = cp.tile([1, 2 * MEDV], F32)
            nc.vector.tensor_copy(out=b2b2[:, 0:MEDV], in_=B("pmc_b2"))
            nc.vector.tensor_copy(out=b2b2[:, MEDV:2 * MEDV], in_=B("pmc_b2"))
            msk = W("masksT")
            HISF = cp.tile([1, NI * 2 * MEDV], F32)

            def his_reduce(t, cg):
                # masked column-sum over j via K=1-out matmuls; psum -> HISF
                nt = 1 if t < NB else 2
                ofs = 2 * MEDV * t
                if nt == 1:
                    ps_his = pa.tile([1, MEDV], F32, tag="acc")
                    nc.tensor.matmul(out=ps_his[:], lhsT=msk[:, t:t + 1],
                                     rhs=cg[:, 0:MEDV], start=True, stop=True)
                    nc.vector.tensor_copy(out=HISF[:, ofs:ofs + MEDV], in_=ps_his[:])
                    nc.vector.memset(HISF[:, ofs + MEDV:ofs + 2 * MEDV], 0.0)
                else:
                    ps_his0 = pa.tile([1, MEDV], F32, tag="acc")
                    nc.tensor.matmul(out=ps_his0[:], lhsT=ones_c[:],
                                     rhs=cg[:, 0:MEDV], start=True, stop=True)
                    ps_his1 = pa.tile([1, MEDV], F32, tag="acc")
                    nc.tensor.matmul(out=ps_his1[:], lhsT=msk[:, t:t + 1],
                                     rhs=cg[:, MEDV:2 * MEDV], start=True, stop=True)
                    nc.vector.tensor_copy(out=HISF[:, ofs:ofs + MEDV], in_=ps_his0[:])
                    nc.vector.tensor_copy(out=HISF[:, ofs + MEDV:ofs + 2 * MEDV], in_=ps_his1[:])

            def make_hT(t):
                nt_ = 1 if t < NB else 2
                hh = []
                for kt in range(2):
                    h = wp.tile([128, V], BF16, tag=f"hT{kt}")
                    nc.scalar.activation(out=h[:, 0:128 * nt_], in_=BTt[kt][:, 0:128 * nt_],
                                         func=TANH, bias=ATb[kt][:, t:t + 1])
                    hh.append(h)
                return hh

            def emit_ct(t, hT):
                # single accumulation group spanning both j-tile regions of one
                # bank: only the first matmul has start=True (bank-wide zero)
                nt = 1 if t < NB else 2
                ps_ct = pp.tile([128, 2 * MEDV], F32, tag="big")
                first = True
                for jt in range(nt):
                    sl = slice(MEDV * jt, MEDV * (jt + 1))
                    for kt in range(2):
                        nc.tensor.matmul(out=ps_ct[:, sl],
                                         lhsT=hT[kt][:, 128 * jt:128 * (jt + 1)],
                                         rhs=w2bf[:, MEDV * kt:MEDV * (kt + 1)],
                                         start=first, stop=False,
                                         skip_group_check=True)
                        first = False
                nc.tensor.matmul(out=ps_ct[:, 0:MEDV * nt], lhsT=ones_r[:],
                                 rhs=b2b2[:, 0:MEDV * nt], start=False, stop=True,
                                 skip_group_check=True)
                return ps_ct

            def emit_tail(t, ps_ct):
                nt = 1 if t < NB else 2
                cont = wp.tile([128, 2 * MEDV], F32, tag="cont")
                nc.scalar.activation(out=cont[:, 0:MEDV * nt], in_=ps_ct[:, 0:MEDV * nt], func=TANH)
                cg = wp.tile([128, 2 * MEDV], F32, tag="cg")
                nc.vector.tensor_mul(out=cg[:, 0:MEDV * nt], in0=cont[:, 0:MEDV * nt],
                                     in1=Gpack[:, 0:MEDV * nt])
                return cg

            # 2-wide software pipeline: two independent chains per stage keep
            # every engine fed across the cross-engine latency hops
            hTs = {0: make_hT(0), 1: make_hT(1)}
            pend = []
            for tb in range(0, NI, 2):
                for u in range(2):
                    if tb + 2 + u < NI:
                        hTs[tb + 2 + u] = make_hT(tb + 2 + u)
                cts = [emit_ct(tb, hTs.pop(tb)), emit_ct(tb + 1, hTs.pop(tb + 1))]
                cgs = [emit_tail(tb, cts[0]), emit_tail(tb + 1, cts[1])]
                for (tp, cgp) in pend:
                    his_reduce(tp, cgp)
                pend = [(tb, cgs[0]), (tb + 1, cgs[1])]
            for (tp, cgp) in pend:
                his_reduce(tp, cgp)

            # ---------- phase 7: prob ----------
            HIS2 = cp.tile([NI, 2 * MEDV], F32)
            dma(out=HIS2[:], in_=HISF[:].rearrange("o (p f) -> o p f", p=NI))
            sv = wp.tile([NI, MEDV], F32, tag="sv")
            nc.vector.tensor_add(out=sv[:], in0=HIS2[:, 0:MEDV], in1=HIS2[:, MEDV:2 * MEDV])
            nc.vector.tensor_add(out=sv[:], in0=sv[:], in1=svp[:])
            prob = cp.tile([NI, MP], F32)
            nc.vector.memset(prob[:], 0.0)
            nc.scalar.activation(out=prob[:, 1:MP], in_=sv[:], func=SIG)
            dma(out=o_prob[:], in_=prob[:])

            # ---------- phase 8: pair_out ----------
            # psum accumulation-group trick: one group spans a bank with
            # disjoint [128, MP] regions; start=True only on the first matmul
            # (its bank-wide zero covers the later regions, which accumulate
            # into zeros).
            prows = cp.tile([1, NI * MP], F32)
            dma(out=prows[:].rearrange("o (p f) -> o p f", p=NI), in_=prob[:])
            pop_all = cp.tile([128, NI * MP], F32)
            pop4 = cp.tile([4, NI * MP], F32)
            TPB = 3   # 3 * 132 f32 = 1584B <= 2KB bank
            CHUNK = 16
            done_upto = 0
            for t0 in range(0, NI, TPB):
                tn = min(TPB, NI - t0)
                po = pp.tile([128, TPB * MP], F32, tag="big")
                po2 = pp.tile([4, TPB * MP], F32, tag="big")
                for i in range(tn):
                    o = (t0 + i) * MP
                    nc.tensor.matmul(out=po[:, i * MP:(i + 1) * MP],
                                     lhsT=prows[0:1, o:o + 128],
                                     rhs=prows[0:1, o:o + MP],
                                     start=(i == 0), stop=(i == tn - 1),
                                     skip_group_check=True)
                    nc.tensor.matmul(out=po2[:, i * MP:(i + 1) * MP],
                                     lhsT=prows[0:1, o + 128:o + MP],
                                     rhs=prows[0:1, o:o + MP],
                                     start=(i == 0), stop=(i == tn - 1),
                                     skip_group_check=True)
                nc.vector.tensor_copy(out=pop_all[:, t0 * MP:(t0 + tn) * MP],
                                      in_=po[:, 0:tn * MP])
                nc.vector.tensor_copy(out=pop4[:, t0 * MP:(t0 + tn) * MP],
                                      in_=po2[:, 0:tn * MP])
                while t0 + tn - done_upto >= CHUNK:
                    e = done_upto + CHUNK
                    dma(out=o_pair[0:128, done_upto:e, :],
                        in_=pop_all[:, done_upto * MP:e * MP].rearrange(
                            "p (t f) -> p t f", t=CHUNK))
                    nc.gpsimd.dma_start(
                        out=o_pair[128:MP, done_upto:e, :],
                        in_=pop4[:, done_upto * MP:e * MP].rearrange(
                            "p (t f) -> p t f", t=CHUNK))
                    done_upto = e
            assert done_upto == NI

            if DBG:
                gdma = nc.gpsimd.dma_start
                gdma(out=DBG["PRown"][:], in_=PRown[:])
                gdma(out=DBG["pr_own"][:], in_=pr_own[:])
                gdma(out=DBG["Gpack"][:], in_=Gpack[:])
                gdma(out=DBG["HIS2"][:], in_=HIS2[:])
                gdma(out=DBG["hm_d"][:], in_=hm_sb["d"][:])
                gdma(out=DBG["mh_dr"][:], in_=mh_own["dr"][:])
                gdma(out=DBG["mh_co"][:], in_=mh_own["co"][:])
                gdma(out=DBG["sv"][:], in_=sv[:])
                gdma(out=DBG["BT0"][:], in_=BTt[0][:])
                gdma(out=DBG["ATb0"][:], in_=ATb[0][:])
                gdma(out=DBG["hmT"][:], in_=hmT[:])

    nc.compile()
    return nc


def _get_nc():
    if "nc" not in _CACHE:
        _CACHE["nc"] = _build_nc()
    return _CACHE["nc"]


def _pack_shared(inputs):
    f32 = lambda x: np.ascontiguousarray(np.asarray(x), dtype=np.float32)
    WL = _wlayout()
    BLL = _blayout()
    CL = _clayout()
    wcrit = np.zeros((128, CL["__total__"][0]), np.float32)
    for nm, arr in (("seg48", _seg_cols(NI * L, L, 1.0 / L)),
                    ("iota131", np.broadcast_to(np.arange(1, MEDV + 1, dtype=np.float32), (NI, MEDV))),
                    ("iota300", np.broadcast_to(np.arange(1, C + 1, dtype=np.float32), (NI, C)))):
        o, r, c = CL[nm]
        wcrit[0:r, o:o + c] = arr
    wpack = np.zeros((128, WL["__total__"][0]), np.float32)

    def put(nm, arr):
        o, r, c = WL[nm]
        arr = np.asarray(arr, np.float32)
        assert arr.shape == (r, c), (nm, arr.shape, (r, c))
        wpack[0:r, o:o + c] = arr

    put("dl2_w1", inputs["dl2_w1"]); put("dl2_w2", inputs["dl2_w2"])
    put("pl2_w1", inputs["pl2_w1"]); put("pl2_w2", inputs["pl2_w2"])
    put("dmb", inputs["diag_med_block"]); put("pmb", inputs["proc_med_block"])
    put("med_block", inputs["med_block"])
    lw1 = f32(inputs["lch_w1"])
    put("lch_w1a", lw1[0:128]); put("lch_w1b", lw1[128:256])
    put("lch_w2", inputs["lch_w2"])
    put("cm_w1", inputs["cm_w1"]); put("cm_w2", inputs["cm_w2"])
    pw1 = f32(inputs["pmc_w1"])
    put("pmc_w1t", pw1[0:128]); put("pmc_w1b", pw1[128:256])
    pw2 = f32(inputs["pmc_w2"])
    put("pmc_w2a", pw2[0:128]); put("pmc_w2b", pw2[128:256])
    cw1 = f32(inputs["clc_w1"])
    ofs = 0
    for kc, wdt in enumerate([128, 3, 128, 128, 44]):
        put(f"clc_w1_{kc}", cw1[ofs:ofs + wdt])
        ofs += wdt
    cw2 = f32(inputs["clc_w2"])
    put("clc_w2_0", cw2[0:128]); put("clc_w2_1", cw2[128:131])
    pb1 = f32(inputs["pmc_b1"]).reshape(256)
    put("pmc_b1", np.stack([pb1[0:128], pb1[128:256]], axis=1))

    bpack = np.zeros((1, BLL["__total__"][0]), np.float32)
    for nm in ("dl2_b1", "dl2_b2", "pl2_b1", "pl2_b2", "clc_b1", "clc_b2",
               "lch_b1", "lch_b2", "pmc_b2", "cm_b1", "cm_b2"):
        o, c = BLL[nm]
        bpack[0, o:o + c] = f32(inputs[nm]).reshape(c)
    return wcrit, wpack, bpack


def _make_in_maps(inputs):
    f32 = lambda x: np.ascontiguousarray(np.asarray(x), dtype=np.float32)
    i32 = lambda x: np.ascontiguousarray(np.asarray(x), dtype=np.int32)

    diag = i32(inputs["diag"]); proc = i32(inputs["proc"])
    drug_ids = i32(inputs["drug_mem_ids"])[0]
    combo_ids = i32(inputs["combo_ids"])[0]
    wcrit_shared, wpack_shared, bpack = _pack_shared(inputs)
    CL = _clayout()

    embcat = np.concatenate([f32(inputs["diag_emb"]), f32(inputs["proc_emb"])], axis=0)
    shared = {
        "bpack": bpack,
        "wpack": wpack_shared,
        "embcat": np.ascontiguousarray(embcat),
    }

    def colmajor_tokens(arr_rows, ntile):
        flat = arr_rows.reshape(-1).astype(np.int32)
        flat = np.concatenate([flat, np.zeros(ntile * 128 - flat.size, np.int32)])
        return flat.reshape(ntile, 128).T.copy()

    in_maps, rows_all = [], []
    mo, _, _ = CL["masksT"]
    for c in range(NCORES):
        rows = _core_rows(c)
        rows_all.append(rows)
        m = dict(shared)
        wc_ = wcrit_shared.copy()
        mk = np.zeros((128, NI), np.float32)
        for t, i in enumerate(rows):
            bt = i // 128
            mk[: i - 128 * bt + 1, t] = 1.0
        wc_[:, mo:mo + NI] = mk
        for nm, arr in (("ids_dr", drug_ids[rows]), ("ids_co", combo_ids[rows])):
            o, r, cc_ = CL[nm]
            wc_[0:r, o:o + cc_] = arr.astype(np.float32)
        m["wcrit"] = wc_
        ix = np.zeros((128, 24), np.int32)
        ix[:, 0:12] = colmajor_tokens(diag[rows], 12)
        ix[:, 12:24] = colmajor_tokens(proc[rows], 12) + DV
        m["ixpack"] = ix
        in_maps.append(m)
    return in_maps, rows_all


def kernel(**inputs):
    from concourse.bass_utils import run_bass_kernel_spmd

    nc = _get_nc()
    in_maps, rows_all = _make_in_maps(inputs)
    res = run_bass_kernel_spmd(nc, in_maps, core_ids=list(range(NCORES)))

    prob = np.zeros((V, MP), np.float32)
    pair = np.zeros((MP, V, MP), np.float32)
    combo = np.zeros((V, CV), np.float32)
    for c in range(NCORES):
        r = res.results[c]
        rows = rows_all[c]
        prob[rows] = r["o_prob"]
        pair[:, rows, :] = r["o_pair"]
        combo[rows] = r["o_combo"]
    return prob, pair, combo
